# revision 9
# baseline (speedup 1.0000x reference)
"""Trainium2 Bass kernel for nn_ABCLayer (ABC-Net style binary conv layer).

Algorithm (algebraically equivalent to the reference, exploiting bilinearity
of convolution):
  - The M=5 binary weight bases use nested thresholds t_m = mean - s_m*std
    (equally spaced, sorted descending), so the Gram matrix G = B @ B.T needed
    for the alpha OLS iteration follows from just 5 threshold counts:
      G[m,n] = 4*min(C_m, C_n) - 2*C_m - 2*C_n + K,  C_m = #{w >= t_m}
    and  b[m] = B @ w = 2*(R_m + t_m*C_m) - sum(w),  R_m = sum(relu(w - t_m)).
  - The 500-step gradient descent a <- a - (LR/K)*(G a - b) is the affine
    iteration a_{t+1} = A a_t + c with A = I - (LR/K) G (symmetric).  We form
    A^500 and S = sum_{i<500} A^i by square-and-multiply (13 small matmuls)
    and get a_500 = A^500 a0 + S c exactly.
  - Conv is linear in weights and input, so
      sum_n beta_n sum_m alpha_m conv(bx_n, bw_m) = conv(xb_eff, w_eff)
    with w_eff = sum_m alpha_m bw_m and xb_eff = sum_n beta_n bx_n.
    One 3x3x128x256 conv over the batch instead of 15.
  - Data parallel: core i processes image i (batch 8 over 8 cores), weight
    preprocessing replicated.  No collectives.

Per-core layouts (host pre-transposes, gather post-transposes):
  wT  : (128, 9, 256)  f32   wT[ci, tap, co] = weight[tap//3, tap%3, ci, co]
  xT  : (128, 32, 32)  f32   xT[ci, r, c]    = x[i, r, c, ci]
  out : (256, 1024)    f32   out[co, r*32+c] = y[i, r, c, co]
"""

import sys

if "/opt/trn_rl_repo" not in sys.path:
    sys.path.insert(0, "/opt/trn_rl_repo")

import numpy as np

import concourse.bass as bass  # noqa: E402
import concourse.tile as tile  # noqa: E402
from concourse import bacc, bass_utils, mybir  # noqa: E402

F32 = mybir.dt.float32
BF16 = mybir.dt.bfloat16
AF = mybir.ActivationFunctionType
OP = mybir.AluOpType

N_CORES = 8
B, H, W, CIN, COUT = 8, 32, 32, 128, 256
M, N = 5, 3
TAPS = 9
K = TAPS * CIN * COUT // TAPS * TAPS  # 294912
K = 3 * 3 * CIN * COUT
LR = 0.01
FD = TAPS * COUT  # 2304 free elements per partition for the weight

_CACHE = {}


def build_nc():
    nc = bacc.Bacc("TRN2", target_bir_lowering=False, debug=False, num_devices=N_CORES)

    wT_d = nc.dram_tensor("wT", (CIN, TAPS, COUT), F32, kind="ExternalInput").ap()
    xT_d = nc.dram_tensor("xT", (CIN, H, W), F32, kind="ExternalInput").ap()
    srow_d = nc.dram_tensor("srow", (1, M), F32, kind="ExternalInput").ap()
    i5_d = nc.dram_tensor("i5", (M, M), F32, kind="ExternalInput").ap()
    shift_d = nc.dram_tensor("shift", (1, N), F32, kind="ExternalInput").ap()
    beta_d = nc.dram_tensor("beta", (1, N), F32, kind="ExternalInput").ap()
    a0_d = nc.dram_tensor("a0", (M, 1), F32, kind="ExternalInput").ap()
    out_d = nc.dram_tensor("out", (COUT, H * W), F32, kind="ExternalOutput").ap()

    with tile.TileContext(nc) as tc:
        with (
            tc.tile_pool(name="big", bufs=1) as big,
            tc.tile_pool(name="scr", bufs=3) as scrp,
            tc.tile_pool(name="sm", bufs=1) as sm,
            tc.tile_pool(name="sps", bufs=4, space="PSUM") as sps,
            tc.tile_pool(name="cps", bufs=1, space="PSUM") as cps,
        ):
            # ---- persistent SBUF tiles ----
            W_sb = big.tile([CIN, TAPS, COUT], F32, tag="W_sb")
            xT_sb = big.tile([CIN, H, W], F32, tag="xT_sb")
            cm = [big.tile([CIN, TAPS, COUT], BF16, tag=f"c{m}", name=f"c{m}") for m in range(M)]
            weff = big.tile([CIN, TAPS, COUT], BF16, tag="weff")
            cx = [big.tile([CIN, H, W], BF16, tag=f"cx{n}", name=f"cx{n}") for n in range(N)]
            cxacc = big.tile([CIN, H, W], F32, tag="cxacc")
            weff32 = big.tile([CIN, TAPS, COUT], F32, tag="weff32")
            xb_pad = big.tile([CIN, H + 2, 36], BF16, tag="xb_pad")
            out_sb = big.tile([128, 2048], F32, tag="out_sb")

            srow_sb = sm.tile([1, M], F32, tag="srow")
            i5_sb = sm.tile([M, M], F32, tag="i5")
            shift_sb = sm.tile([1, N], F32, tag="shift")
            beta_sb = sm.tile([1, N], F32, tag="beta")
            a0_sb = sm.tile([M, 1], F32, tag="a0")
            ones128 = sm.tile([128, 1], F32, tag="ones128")
            onesr = sm.tile([1, 128], F32, tag="onesr")
            ssum = sm.tile([128, 1], F32, tag="ssum")
            ssq = sm.tile([128, 1], F32, tag="ssq")
            cacc = sm.tile([128, M], F32, tag="cacc")
            racc = sm.tile([128, M], F32, tag="racc")
            row_w = sm.tile([1, 2 * M], F32, tag="row_w")
            row_x = sm.tile([1, 2 * N + 1], F32, tag="row_x")
            bcw = sm.tile([128, 2 * M], F32, tag="bcw")
            bcx = sm.tile([128, 2 * N + 1], F32, tag="bcx")
            mean_sb = sm.tile([1, 1], F32, tag="mean")
            var_sb = sm.tile([1, 1], F32, tag="var")
            m2_sb = sm.tile([1, 1], F32, tag="m2")
            sig_sb = sm.tile([1, 1], F32, tag="sig")
            negsig = sm.tile([1, 1], F32, tag="negsig")
            heron_r = sm.tile([1, 1], F32, tag="heron_r")
            heron_p = sm.tile([1, 1], F32, tag="heron_p")
            swlr = sm.tile([1, 1], F32, tag="swlr")
            redC = sm.tile([1, M], F32, tag="redC")
            redR = sm.tile([1, M], F32, tag="redR")
            crows = sm.tile([M, M], F32, tag="crows")
            ccols = sm.tile([M, M], F32, tag="ccols")
            gmin = sm.tile([M, M], F32, tag="gmin")
            csum = sm.tile([M, M], F32, tag="csum")
            t1_sb = sm.tile([M, M], F32, tag="t1")
            A5 = sm.tile([M, M], F32, tag="A5")
            v1_sb = sm.tile([1, M], F32, tag="v1")
            v2_sb = sm.tile([1, M], F32, tag="v2")
            cvrow = sm.tile([1, M], F32, tag="cvrow")
            cvec_sb = sm.tile([M, 1], F32, tag="cvec")
            SP = sm.tile([M, 2 * M], F32, tag="SP")
            acol_sb = sm.tile([M, 1], F32, tag="acol")
            arow6 = sm.tile([1, M + 1], F32, tag="arow6")
            ab_sb = sm.tile([128, M + 1], F32, tag="ab")

            # ---- input DMAs ----
            nc.sync.dma_start(out=W_sb[:, :, :], in_=wT_d[:, :, :])
            nc.sync.dma_start(out=xT_sb[:, :, :], in_=xT_d[:, :, :])
            nc.sync.dma_start(out=srow_sb[:, :], in_=srow_d[:, :])
            nc.sync.dma_start(out=i5_sb[:, :], in_=i5_d[:, :])
            nc.sync.dma_start(out=shift_sb[:, :], in_=shift_d[:, :])
            nc.sync.dma_start(out=beta_sb[:, :], in_=beta_d[:, :])
            nc.sync.dma_start(out=a0_sb[:, :], in_=a0_d[:, :])

            nc.vector.memset(ones128[:, :], 1.0)
            nc.vector.memset(onesr[:, :], 1.0)
            nc.vector.memset(xb_pad[:, :, :], 0.0)

            # ---- x-side thresholds (independent of weight stats) ----
            # row_x = [0.5 - shift (3) | 2*beta (3) | sum(beta) (1)]
            nc.vector.tensor_scalar(
                out=row_x[:, 0:N], in0=shift_sb[:, :], scalar1=-1.0, scalar2=0.5,
                op0=OP.mult, op1=OP.add)
            nc.vector.tensor_scalar(
                out=row_x[:, N:2 * N], in0=beta_sb[:, :], scalar1=2.0, scalar2=None,
                op0=OP.mult)
            nc.vector.tensor_reduce(
                out=row_x[:, 2 * N:2 * N + 1], in_=beta_sb[:, :],
                axis=mybir.AxisListType.X, op=OP.add)
            bcx_ps = sps.tile([128, 2 * N + 1], F32, tag="sps")
            nc.tensor.matmul(bcx_ps[:, :], onesr[:, :], row_x[:, :])
            nc.vector.tensor_copy(bcx[:, :], bcx_ps[:, :])

            # ---- x binarization: xb_eff = sum_n beta_n * sign(x - (0.5-shift_n))
            for n in range(N):
                nc.vector.tensor_scalar(
                    out=cx[n][:, :, :], in0=xT_sb[:, :, :],
                    scalar1=bcx[:, n:n + 1], scalar2=None, op0=OP.is_ge)
            nc.vector.tensor_scalar(
                out=cxacc[:, :, :], in0=cx[0][:, :, :],
                scalar1=bcx[:, N:N + 1], scalar2=bcx[:, 2 * N:2 * N + 1],
                op0=OP.mult, op1=OP.subtract)
            nc.vector.scalar_tensor_tensor(
                out=cxacc[:, :, :], in0=cx[1][:, :, :],
                scalar=bcx[:, N + 1:N + 2], in1=cxacc[:, :, :],
                op0=OP.mult, op1=OP.add)
            # last accumulation writes the padded bf16 tile directly (single
            # rounding; the f32 chain avoids compounded bf16 error)
            nc.vector.scalar_tensor_tensor(
                out=xb_pad[:, 1:H + 1, 2:W + 2], in0=cx[2][:, :, :],
                scalar=bcx[:, N + 2:N + 3], in1=cxacc[:, :, :],
                op0=OP.mult, op1=OP.add)

            # ---- weight stats: sum (DVE) and sum-of-squares (ACT) ----
            s_scr = scrp.tile([CIN, TAPS, COUT], F32, tag="scr")
            nc.vector.tensor_scalar(
                out=s_scr[:, :, :], in0=W_sb[:, :, :], scalar1=1.0, scalar2=None,
                op0=OP.mult, op1=OP.add, accum_out=ssum[:, :])
            q_scr = scrp.tile([CIN, TAPS, COUT], F32, tag="scr")
            nc.scalar.activation(
                q_scr[:, :, :], W_sb[:, :, :], AF.Square, accum_out=ssq[:, :])

            s1_ps = sps.tile([1, 1], F32, tag="sps")
            nc.tensor.matmul(s1_ps[:, :], ones128[:, :], ssum[:, :])
            s2_ps = sps.tile([1, 1], F32, tag="sps")
            nc.tensor.matmul(s2_ps[:, :], ones128[:, :], ssq[:, :])

            # mean, var, std (std via 3 Heron iterations seeded at 0.05)
            nc.vector.tensor_scalar(
                out=mean_sb[:, :], in0=s1_ps[:, :], scalar1=1.0 / K, scalar2=None,
                op0=OP.mult)
            nc.vector.tensor_mul(m2_sb[:, :], mean_sb[:, :], mean_sb[:, :])
            nc.vector.scalar_tensor_tensor(
                out=var_sb[:, :], in0=s2_ps[:, :], scalar=1.0 / K, in1=m2_sb[:, :],
                op0=OP.mult, op1=OP.subtract)
            nc.vector.memset(sig_sb[:, :], 0.05)
            for _ in range(3):
                nc.vector.reciprocal(heron_p[:, :], sig_sb[:, :])
                nc.vector.tensor_mul(heron_r[:, :], var_sb[:, :], heron_p[:, :])
                nc.vector.tensor_add(sig_sb[:, :], sig_sb[:, :], heron_r[:, :])
                nc.vector.tensor_scalar(
                    out=sig_sb[:, :], in0=sig_sb[:, :], scalar1=0.5, scalar2=None,
                    op0=OP.mult)

            # thresholds: t_m = mean - s_m * sig ; row_w = [t (5) | -t (5)]
            nc.vector.tensor_scalar(
                out=negsig[:, :], in0=sig_sb[:, :], scalar1=-1.0, scalar2=None,
                op0=OP.mult)
            nc.vector.tensor_scalar(
                out=row_w[:, 0:M], in0=srow_sb[:, :], scalar1=negsig[:, :],
                scalar2=mean_sb[:, :], op0=OP.mult, op1=OP.add)
            nc.vector.tensor_scalar(
                out=row_w[:, M:2 * M], in0=row_w[:, 0:M], scalar1=-1.0, scalar2=None,
                op0=OP.mult)
            bcw_ps = sps.tile([128, 2 * M], F32, tag="sps")
            nc.tensor.matmul(bcw_ps[:, :], onesr[:, :], row_w[:, :])
            nc.vector.tensor_copy(bcw[:, :], bcw_ps[:, :])

            # ---- masks c_m = (w >= t_m) with fused counts; relu sums on ACT ----
            for m in range(M):
                nc.vector.tensor_scalar(
                    out=cm[m][:, :, :], in0=W_sb[:, :, :],
                    scalar1=bcw[:, m:m + 1], scalar2=None, op0=OP.is_ge,
                    op1=OP.add, accum_out=cacc[:, m:m + 1])
            for m in range(M):
                r_scr = scrp.tile([CIN, TAPS, COUT], F32, tag="scr")
                nc.scalar.activation(
                    r_scr[:, :, :], W_sb[:, :, :], AF.Relu,
                    bias=bcw[:, M + m:M + m + 1], accum_out=racc[:, m:m + 1])

            redC_ps = sps.tile([1, M], F32, tag="sps")
            nc.tensor.matmul(redC_ps[:, :], ones128[:, :], cacc[:, :])
            nc.vector.tensor_copy(redC[:, :], redC_ps[:, :])
            redR_ps = sps.tile([1, M], F32, tag="sps")
            nc.tensor.matmul(redR_ps[:, :], ones128[:, :], racc[:, :])
            nc.vector.tensor_copy(redR[:, :], redR_ps[:, :])

            # ---- G and A = I - (LR/K) G ----
            cr_ps = sps.tile([M, M], F32, tag="sps")
            nc.tensor.matmul(cr_ps[:, :], onesr[:, 0:M], redC[:, :])
            nc.vector.tensor_copy(crows[:, :], cr_ps[:, :])
            cc_ps = sps.tile([M, M], F32, tag="sps")
            nc.tensor.matmul(cc_ps[:, :], redC[:, :], onesr[:, 0:M])
            nc.vector.tensor_copy(ccols[:, :], cc_ps[:, :])
            nc.vector.tensor_tensor(
                out=gmin[:, :], in0=crows[:, :], in1=ccols[:, :], op=OP.min)
            nc.vector.tensor_add(csum[:, :], crows[:, :], ccols[:, :])
            # A = I - (4 LR/K) gmin + (2 LR/K) csum - LR
            nc.vector.scalar_tensor_tensor(
                out=t1_sb[:, :], in0=gmin[:, :], scalar=-4.0 * LR / K,
                in1=i5_sb[:, :], op0=OP.mult, op1=OP.add)
            nc.vector.scalar_tensor_tensor(
                out=A5[:, :], in0=csum[:, :], scalar=2.0 * LR / K, in1=t1_sb[:, :],
                op0=OP.mult, op1=OP.add)
            nc.vector.tensor_scalar(
                out=A5[:, :], in0=A5[:, :], scalar1=LR, scalar2=None,
                op0=OP.subtract)

            # ---- c = (LR/K) b ;  b_m = 2 (R_m + t_m C_m) - sum(w) ----
            nc.vector.tensor_scalar(
                out=swlr[:, :], in0=s1_ps[:, :], scalar1=LR / K, scalar2=None,
                op0=OP.mult)
            nc.vector.tensor_mul(v1_sb[:, :], redC[:, :], row_w[:, 0:M])
            nc.vector.tensor_add(v2_sb[:, :], redR[:, :], v1_sb[:, :])
            nc.vector.tensor_scalar(
                out=cvrow[:, :], in0=v2_sb[:, :], scalar1=2.0 * LR / K,
                scalar2=swlr[:, :], op0=OP.mult, op1=OP.subtract)
            cv_ps = sps.tile([M, 1], F32, tag="sps")
            nc.tensor.matmul(cv_ps[:, :], cvrow[:, :], onesr[:, 0:1])
            nc.vector.tensor_copy(cvec_sb[:, :], cv_ps[:, :])

            # ---- a_500 = A^500 a0 + (sum_{i<500} A^i) c by square-and-multiply
            # SP = [S | P]; after MSB of 500=0b111110100: P=A, S=I
            nc.vector.tensor_copy(SP[:, 0:M], i5_sb[:, :])
            nc.vector.tensor_copy(SP[:, M:2 * M], A5[:, :])
            for bit in (1, 1, 1, 1, 0, 1, 0, 0):
                d_ps = sps.tile([M, 2 * M], F32, tag="sps")
                nc.tensor.matmul(d_ps[:, :], SP[:, M:2 * M], SP[:, :])
                nc.vector.tensor_add(SP[:, 0:M], SP[:, 0:M], d_ps[:, 0:M])
                nc.vector.tensor_copy(SP[:, M:2 * M], d_ps[:, M:2 * M])
                if bit:
                    nc.vector.tensor_add(SP[:, 0:M], SP[:, 0:M], SP[:, M:2 * M])
                    p_ps = sps.tile([M, M], F32, tag="sps")
                    nc.tensor.matmul(p_ps[:, :], SP[:, M:2 * M], A5[:, :])
                    nc.vector.tensor_copy(SP[:, M:2 * M], p_ps[:, :])

            a_ps = sps.tile([M, 1], F32, tag="sps")
            nc.tensor.matmul(a_ps[:, :], SP[:, M:2 * M], a0_sb[:, :],
                             start=True, stop=False)
            nc.tensor.matmul(a_ps[:, :], SP[:, 0:M], cvec_sb[:, :],
                             start=False, stop=True)
            nc.vector.tensor_copy(acol_sb[:, :], a_ps[:, :])
            ar_ps = sps.tile([1, M], F32, tag="sps")
            nc.tensor.matmul(ar_ps[:, :], acol_sb[:, :], i5_sb[:, :])
            # arow6 = [2*alpha (5) | sum(alpha) (1)]
            nc.vector.tensor_scalar(
                out=arow6[:, 0:M], in0=ar_ps[:, :], scalar1=2.0, scalar2=None,
                op0=OP.mult)
            nc.vector.tensor_reduce(
                out=arow6[:, M:M + 1], in_=ar_ps[:, :],
                axis=mybir.AxisListType.X, op=OP.add)
            b6_ps = sps.tile([128, M + 1], F32, tag="sps")
            nc.tensor.matmul(b6_ps[:, :], onesr[:, :], arow6[:, :])
            nc.vector.tensor_copy(ab_sb[:, :], b6_ps[:, :])

            # ---- w_eff = sum_m 2 alpha_m c_m - sum(alpha) ----
            # f32 accumulation; final step rounds once into the bf16 tile
            nc.vector.tensor_scalar(
                out=weff32[:, :, :], in0=cm[0][:, :, :], scalar1=ab_sb[:, 0:1],
                scalar2=ab_sb[:, M:M + 1], op0=OP.mult, op1=OP.subtract)
            for m in range(1, M - 1):
                nc.vector.scalar_tensor_tensor(
                    out=weff32[:, :, :], in0=cm[m][:, :, :],
                    scalar=ab_sb[:, m:m + 1], in1=weff32[:, :, :],
                    op0=OP.mult, op1=OP.add)
            nc.vector.scalar_tensor_tensor(
                out=weff[:, :, :], in0=cm[M - 1][:, :, :],
                scalar=ab_sb[:, M - 1:M], in1=weff32[:, :, :],
                op0=OP.mult, op1=OP.add)

            # ---- conv: out[co, pos] += w_eff[ci, tap, co].T @ xb[ci, pos(tap)]
            pc = [[cps.tile([128, 512], F32, tag=f"pc{c}_{t}", name=f"pc{c}_{t}") for t in range(2)]
                  for c in range(2)]
            for ch in range(2):
                for tap in range(TAPS):
                    dy, dx = tap // 3, tap % 3
                    lhs = weff[:, tap, ch * 128:(ch + 1) * 128]
                    for t in range(2):
                        r0 = t * 16
                        rhs = xb_pad[:, dy + r0:dy + r0 + 16, dx + 1:dx + 1 + W]
                        nc.tensor.matmul(
                            pc[ch][t][:, :], lhs, rhs,
                            start=(tap == 0), stop=(tap == TAPS - 1))
            for ch in range(2):
                for t in range(2):
                    dst = out_sb[:, (ch * 2 + t) * 512:(ch * 2 + t + 1) * 512]
                    if t == 0:
                        nc.scalar.copy(dst, pc[ch][t][:, :])
                    else:
                        nc.vector.tensor_copy(dst, pc[ch][t][:, :])
            nc.sync.dma_start(out=out_d[0:128, :], in_=out_sb[:, 0:1024])
            nc.sync.dma_start(out=out_d[128:256, :], in_=out_sb[:, 1024:2048])

    nc.compile()
    return nc


def make_in_maps(x, weight, shiftPara, beta, alphas_init):
    wT = np.ascontiguousarray(
        weight.reshape(TAPS, CIN, COUT).transpose(1, 0, 2)).astype(np.float32)
    srow = (-1.0 + np.arange(M, dtype=np.float32) * (2.0 / (M - 1))).reshape(1, M)
    i5 = np.eye(M, dtype=np.float32)
    shift_in = np.asarray(shiftPara, np.float32).reshape(1, N)
    beta_in = np.asarray(beta, np.float32).reshape(1, N)
    a0 = np.asarray(alphas_init, np.float32).reshape(M, 1)
    in_maps = []
    for i in range(N_CORES):
        xT = np.ascontiguousarray(
            x[i].reshape(H * W, CIN).T).reshape(CIN, H, W).astype(np.float32)
        in_maps.append({
            "wT": wT, "xT": xT, "srow": srow, "i5": i5,
            "shift": shift_in, "beta": beta_in, "a0": a0,
        })
    return in_maps


def kernel(x, weight, shiftPara, beta, alphas_init):
    if "nc" not in _CACHE:
        _CACHE["nc"] = build_nc()
    nc = _CACHE["nc"]
    in_maps = make_in_maps(x, weight, shiftPara, beta, alphas_init)
    res = bass_utils.run_bass_kernel_spmd(
        nc, in_maps, core_ids=list(range(N_CORES)))
    outs = [res.results[i]["out"] for i in range(N_CORES)]
    out = np.stack(outs, axis=0)  # (8, 256, 1024)
    out = out.transpose(0, 2, 1).reshape(B, H, W, COUT)
    return np.ascontiguousarray(out).astype(np.float32)
